# revision 25
# baseline (speedup 1.0000x reference)
"""Multi-head attention 3D kernel for Trainium2, 8 NeuronCores.

Hybrid: bf16 row-banded scores; fp8 DoubleRow attn@V for head 0, bf16
attn@V for head 1.

Problem: x[2, 256, 16, 16, 16] -> MHA(8 heads, head_dim 32) over N=4096
tokens per batch, QKV projection, softmax attention, out projection, bias,
residual.

Sharding: 8 cores = 2 batches x 4 head-pairs; each core computes the full
attention for its batch and 2 heads, emitting a partial [4096, 256] bf16
output; the host sums partials, adds bias + residual.

Device-side design (per core):
  - Phase A (interleaved with the first q-chunk): Q^T/K^T projected in
    bf16 -> PSUM, evicted to band-replicated SBUF tiles [128, n_tok]
    (partitions 32r..32r+31 = head r//2), V' tiles per head with a ones
    column for the softmax denominator (V_h0 in fp8 * sv, V_h1 in bf16).
  - Scores S^T[k, q]: 4 bf16 matmuls per k-group (2 heads x 2 k-tiles)
    in distinct 32-row PE bands via tile_position (they overlap on HW),
    32-dim contraction, PSUM [128, 2, 512] per (k-group, head),
    triple-buffered so the PE|exp pipeline flows.
  - exp(): GPSIMD cannot read PSUM (and its casts run ~4 cyc/elem, far
    too slow to help), so exp splits between Scalar (real Exp) and DVE
    (bf16-bit-domain Schraudolph, +-4% sawtooth averaging out over
    4096-term sums):
      head 0 -> Scalar with fp8 output; attn@V is a DoubleRow fp8
        matmul writing PSUM partitions 0:64 (DR requires out base 0):
        row 0 = softmax denominator (ones column in V'), rows 1:33 = O'^T.
      head 1 -> DVE with bf16 output; attn@V is two bf16 matmuls at
        tile_position (0,64) writing partitions 64:97 of the SAME PSUM
        bank: row 64 = denominator, 65:97 = O'^T.
    attn@V is emitted one k-group behind its exp (software pipelining).
  - Out-projection bf16 with denominator passthrough row; reciprocal +
    per-token multiply (DVE); head partials summed on GPSIMD (SBUF-only
    engine); y stored bf16.

fp8 scale sv and the softmax shift are computed host-side from the actual
inputs (with margin) and folded into weights / exp constants.

Measured on HW (neuron-profile): 258 us per core; engines: PE 83%,
DVE 82%, Scalar 63% busy. Rejected redesigns (measured slower): routing
h1 exp through GpSimd bf16->fp8 casts for DoubleRow attn@V (GpSimd CAST
is 3.6 us per [128,1024] — 4.2 cyc/elem — making it the bottleneck at
439 us and deepening PE p-state throttling), and per-head-sequential
k-group sweeps (ring-depth/PSUM trade-offs; sim 224 us but HW-worse).
"""

import math

import ml_dtypes
import numpy as np

import concourse.bass as bass
import concourse.tile as tile
from concourse import mybir
from concourse.bass_utils import run_bass_kernel_spmd

F32 = mybir.dt.float32
BF16 = mybir.dt.bfloat16
F8 = mybir.dt.float8e4
I16 = mybir.dt.int16

A16 = 128.0 / math.log(2.0)  # bf16-bit-domain Schraudolph scale
B16 = 127.0 * 128.0 - 7.0

EMBED = 256
HEADS = 8
HD = 32
B = 2
N_TOK = 4096


ACTB_RATIO = 0.0  # fraction of h1 k-groups shifted DVE -> ACT (bf16 out)


def default_exp_split(qc, kg, h):
    """h0 -> Scalar/fp8 (feeds DoubleRow attnV); h1 -> mostly DVE bf16
    Schraudolph, with an ACTB_RATIO share Bresenham-shifted to the Scalar
    engine (bf16 out) to balance the HW profile: DVE measured 82% busy
    (215us) vs Scalar 63% (165us) on the all-DVE-h1 split.
    """
    if h == 0:
        return "act"
    u = qc * 16 + kg
    r = ACTB_RATIO
    return "actb" if int((u + 1) * r) > int(u * r) else "dveb"


def build_nc(n_tok: int = N_TOK, reps: int = 1, se: float = 1.0,
             off: float = 0.0, exp_split=None, ablate: str = "",
             split_waits: bool = True) -> bass.Bass:
    """Build the single-core Bass program (same program on all 8 cores)."""
    assert n_tok % 512 == 0
    n_qc = n_tok // 512
    n_kg = n_tok // 256
    n_kt = n_tok // 128
    DR = mybir.MatmulPerfMode.DoubleRow

    if exp_split is None:
        exp_split = default_exp_split

    nc = bass.Bass()
    xT = nc.declare_dram_parameter("xT", [EMBED, n_tok], BF16, isOutput=False)
    # band-replicated Q/K weights (baseline layout): bands 0,1 = head0,
    # bands 2,3 = head1 of the 128 columns.
    wq4 = nc.declare_dram_parameter("wq4", [EMBED, 128], BF16, isOutput=False)
    wk4 = nc.declare_dram_parameter("wk4", [EMBED, 128], BF16, isOutput=False)
    wv2 = nc.declare_dram_parameter("wv2", [2, 128, 64], BF16, isOutput=False)
    wo0 = nc.declare_dram_parameter("wo0", [33, 258], BF16, isOutput=False)
    wo1 = nc.declare_dram_parameter("wo1", [33, 258], BF16, isOutput=False)
    y = nc.declare_dram_parameter("y", [n_tok, EMBED], BF16, isOutput=True)
    build_body(nc, xT, wq4, wk4, wv2, wo0, wo1, y, n_tok=n_tok, reps=reps,
               se=se, off=off, exp_split=exp_split, ablate=ablate)
    if split_waits:
        _split_multi_waits(nc)
    return nc


def build_body(nc, xT, wq4, wk4, wv2, wo0, wo1, y, *, n_tok, reps=1, se=1.0,
               off=0.0, exp_split=None, ablate=""):
    n_qc = n_tok // 512
    n_kg = n_tok // 256
    n_kt = n_tok // 128
    DR = mybir.MatmulPerfMode.DoubleRow
    if exp_split is None:
        exp_split = default_exp_split

    t1 = A16 * se              # bf16 Schraudolph: bits = t1*s' + t2
    t2 = B16 - A16 * off

    with tile.TileContext(nc) as tc:
        with (
            tc.tile_pool(name="consts", bufs=1) as consts,
            tc.tile_pool(name="pQK", bufs=2) as pQK,
            tc.tile_pool(name="pP", bufs=7) as pP,
            tc.tile_pool(name="pOsb", bufs=2) as pOsb,
            tc.tile_pool(name="pY", bufs=6) as pY,
            tc.tile_pool(name="pR", bufs=4) as pR,
            tc.tile_pool(name="psS", bufs=3, space="PSUM") as psS,
            tc.tile_pool(name="psO", bufs=2, space="PSUM") as psO,
        ):
            import contextlib
            rep_loop = (
                tc.For_i(0, reps, 1, hint_engines=(
                    mybir.EngineType.PE, mybir.EngineType.Activation,
                    mybir.EngineType.DVE, mybir.EngineType.SP,
                ))
                if reps > 1 else contextlib.nullcontext()
            )
            with rep_loop:
                # ---- consts / inputs ----
                xt = []
                for c in range(2):
                    t = consts.tile([128, n_tok], BF16, tag=f"xt{c}")
                    xt.append(t)
                wq4s, wk4s = [], []
                for c in range(2):
                    tq = consts.tile([128, 128], BF16, tag=f"wq{c}", name=f"wq{c}")
                    nc.sync.dma_start(out=tq, in_=wq4[c * 128 : (c + 1) * 128, :])
                    wq4s.append(tq)
                    tk = consts.tile([128, 128], BF16, tag=f"wk{c}", name=f"wk{c}")
                    nc.sync.dma_start(out=tk, in_=wk4[c * 128 : (c + 1) * 128, :])
                    wk4s.append(tk)
                wv2s = []
                for c in range(2):
                    t = consts.tile([128, 64], BF16, tag=f"wv{c}")
                    nc.sync.dma_start(out=t, in_=wv2[c, :, :])
                    wv2s.append(t)
                wos = []
                for h, wo in enumerate((wo0, wo1)):
                    t = consts.tile([33, 258], BF16, tag=f"wo{h}")
                    nc.sync.dma_start(out=t, in_=wo[:, :])
                    wos.append(t)
                bias_t = consts.tile([128, 1], F32, tag="bias")
                nc.vector.memset(bias_t, -off)

                # bf16 Q^T/K^T band-replicated: [128, n_tok]; partitions
                # 32r..32r+31 hold head (r//2)'s Q^T/K^T (r in 0..3).
                QT4 = consts.tile([128, n_tok], BF16, tag="QT4")
                KT4 = consts.tile([128, n_tok], BF16, tag="KT4")
                V2bf = consts.tile([128, n_kt, 33], BF16, tag="V2bf")
                nc.gpsimd.memset(V2bf, 0.0)
                nc.gpsimd.memset(V2bf[:, :, 0:1], 1.0)
                V2p8 = consts.tile([128, n_kt, 64], F8, tag="V2p8")
                nc.gpsimd.memset(V2p8, 0.0)
                nc.gpsimd.memset(V2p8[:, :, 0:1], 1.0)

                def phase_a_chunk(j):
                    js = slice(j * 512, (j + 1) * 512)
                    for c in range(2):
                        nc.sync.dma_start(out=xt[c][:, js], in_=xT[c * 128 : (c + 1) * 128, js])
                    for dst, w in ((QT4, wq4s), (KT4, wk4s)):
                        ps = psS.tile([128, 512], F32, tag="ps_big", name="ps_qk")
                        nc.tensor.matmul(ps, lhsT=w[0], rhs=xt[0][:, js],
                                         start=True, stop=False)
                        nc.tensor.matmul(ps, lhsT=w[1], rhs=xt[1][:, js],
                                         start=False, stop=True)
                        (nc.vector.tensor_copy if dst is QT4 else nc.scalar.copy)(dst[:, js], ps)
                    for kt in range(4 * j, 4 * j + 4):
                        produce_v(kt)

                def produce_v(kt):
                    ps_v = psS.tile([128, 64], F32, tag="ps_big", name="ps_v")
                    nc.tensor.matmul(
                        ps_v, lhsT=xt[0][:, kt * 128 : (kt + 1) * 128], rhs=wv2s[0],
                        start=True, stop=False,
                    )
                    nc.tensor.matmul(
                        ps_v, lhsT=xt[1][:, kt * 128 : (kt + 1) * 128], rhs=wv2s[1],
                        start=False, stop=True,
                    )
                    nc.vector.tensor_copy(V2p8[:, kt, 1:33], ps_v[:, 0:32])
                    nc.vector.tensor_copy(V2bf[:, kt, 1:33], ps_v[:, 32:64])

                def emit_attnv(qc, kg, pOb, P8, engs):
                    nc.tensor.matmul(
                        pOb[0:64, :],
                        lhsT=V2p8[:, 2 * kg : 2 * kg + 2, :],
                        rhs=P8[0],
                        start=(kg == 0), stop=(kg == n_kg - 1),
                        perf_mode=DR,
                        skip_group_check=True,
                    )
                    for i in range(2):
                        kt = 2 * kg + i
                        nc.tensor.matmul(
                            pOb[64:97, :],
                            lhsT=V2bf[:, kt, :],
                            rhs=P8[1][:, i, :],
                            start=(kg == 0 and i == 0),
                            stop=(kg == n_kg - 1 and i == 1),
                            tile_position=(0, 64),
                            skip_group_check=True,
                        )

                # attnV is emitted one k-group behind its exp so the PE never
                # waits on the exp engines (software pipelining).
                av_q = []

                def do_kg(qc, kg, pOb):
                    pS = [
                        psS.tile([128, 2, 512], F32, tag="ps_big", name=f"pS{_h}")
                        for _h in range(2)
                    ]
                    qs = slice(qc * 512, (qc + 1) * 512)
                    for h in range(2):
                        for i in range(2):
                            r = 2 * h + i
                            kt = 2 * kg + i
                            nc.tensor.matmul(
                                pS[h][:, i, :],
                                lhsT=KT4[32 * r : 32 * r + 32,
                                         kt * 128 : (kt + 1) * 128],
                                rhs=QT4[32 * r : 32 * r + 32, qs],
                                start=True, stop=True,
                                tile_position=(32 * r, 0),
                            )
                    engs = [exp_split(qc, kg, h) for h in range(2)]
                    P8 = [
                        pP.tile(
                            [128, 2, 512], F8 if engs[_h] == "act" else BF16,
                            tag="p8", name=f"P8_{_h}",
                        )
                        for _h in range(2)
                    ]
                    for h in range(2):
                        if "exp" in ablate:
                            break
                        if engs[h] in ("act", "actb"):
                            nc.scalar.activation(
                                P8[h], pS[h], mybir.ActivationFunctionType.Exp,
                                bias=bias_t[:, 0:1], scale=se,
                            )
                        else:
                            nc.vector.tensor_scalar(
                                out=P8[h].bitcast(I16), in0=pS[h],
                                scalar1=t1, scalar2=t2,
                                op0=mybir.AluOpType.mult, op1=mybir.AluOpType.add,
                            )
                    if "attnv" not in ablate:
                        while av_q:
                            emit_attnv(*av_q.pop(0))
                        av_q.append((qc, kg, pOb, P8, engs))

                def qc_epilogue(qc, pOb):
                    if "epi" in ablate:
                        return
                    Osb = [
                        pOsb.tile([33, 512], BF16, tag=f"osb{_h}", name=f"Osb{_h}")
                        for _h in range(2)
                    ]
                    nc.vector.tensor_copy(Osb[0], pOb[0:33, :])
                    nc.scalar.copy(Osb[1], pOb[64:97, :])
                    for t in range(4):
                        qt = qc * 4 + t
                        ts = slice(t * 128, (t + 1) * 128)
                        yh = []
                        for h in range(2):
                            psy = psO.tile(
                                [128, 258], F32, tag="po", name=f"psy{h}"
                            )
                            nc.tensor.matmul(
                                psy, lhsT=Osb[h][:, ts], rhs=wos[h],
                                start=True, stop=True,
                            )
                            rden = pR.tile([128, 1], F32, tag="rden")
                            nc.vector.reciprocal(rden, psy[:, 256:257])
                            ysb = pY.tile([128, 256], BF16, tag="ysb")
                            nc.vector.tensor_scalar_mul(ysb, psy[:, 0:256], rden)
                            yh.append(ysb)
                        yout = pY.tile([128, 256], BF16, tag="ysb")
                        nc.gpsimd.tensor_add(yout, yh[0], yh[1])
                        nc.sync.dma_start(out=y[qt * 128 : (qt + 1) * 128, :], in_=yout)

                pending = None
                for qc in range(n_qc):
                    pOb = psO.tile([128, 512], F32, tag="po", name="pOb")
                    if qc == 0:
                        for j in range(n_tok // 512):
                            phase_a_chunk(j)
                            for kg in (2 * j, 2 * j + 1):
                                do_kg(0, kg, pOb)
                                if kg == 4 and pending is not None:
                                    qc_epilogue(*pending)
                                    pending = None
                    else:
                        for kg in range(n_kg):
                            do_kg(qc, kg, pOb)
                            if kg == 4 and pending is not None:
                                qc_epilogue(*pending)
                                pending = None
                    pending = (qc, pOb)
                while av_q:
                    emit_attnv(*av_q.pop(0))
                qc_epilogue(*pending)


def _split_multi_waits(nc, max_waits: int = 1):
    """Walrus accepts at most one sync wait per instruction; spill extras
    onto single-wait NoOps placed just before."""
    for f in nc.m.functions:
        for bb in f.blocks:
            new = []
            for ins in bb.instructions:
                si = ins.sync_info
                if si is not None and si.on_wait and len(si.on_wait) > max_waits:
                    waits = list(si.on_wait)
                    keep, spill = waits[-max_waits:], waits[:-max_waits]
                    for i, w in enumerate(spill):
                        new.append(
                            mybir.InstNoOp(
                                name=f"{ins.name}-w{i}",
                                engine=ins.engine,
                                ins=[], outs=[],
                                debug=ins.debug,
                                sync_info=mybir.SyncInfo(on_wait=[w], on_update=[]),
                            )
                        )
                    ins.sync_info = mybir.SyncInfo(
                        on_wait=keep, on_update=list(si.on_update or [])
                    )
                new.append(ins)
            bb.instructions = new


def _prep(x, W_qkv, W_out, n_tok):
    """Host-side: scales from data, shared across all cores."""
    x = np.asarray(x, dtype=np.float32)
    W_qkv = np.asarray(W_qkv, dtype=np.float32)
    W_out = np.asarray(W_out, dtype=np.float32)
    Bb, C = x.shape[0], x.shape[1]
    xf = x.reshape(Bb, C, -1)[:, :, :n_tok]  # [B, C, N]
    Wq, Wk, Wv = W_qkv[:, 0:EMBED], W_qkv[:, EMBED : 2 * EMBED], W_qkv[:, 2 * EMBED :]
    scale = 1.0 / math.sqrt(HD)

    Q = np.einsum("cd,bct->bdt", Wq, xf) * scale
    K = np.einsum("cd,bct->bdt", Wk, xf)
    V = np.einsum("cd,bct->bdt", Wv, xf)
    sv = 224.0 / max(np.abs(V).max(), 1e-30)
    # score upper bound per head: max ||q|| * max ||k|| (Cauchy-Schwarz)
    smax = 0.0
    for b in range(Bb):
        for h in range(HEADS):
            qn = np.linalg.norm(Q[b, h * HD : (h + 1) * HD, :], axis=0).max()
            kn = np.linalg.norm(K[b, h * HD : (h + 1) * HD, :], axis=0).max()
            smax = max(smax, float(qn * kn))
    off = float(max(0.0, smax - 4.5))
    se = 1.0  # scores are bf16 natural-scale (1/sqrt(32) folded into wq4)
    sv = float(sv)
    return dict(x=x, xf=xf, Wq=Wq, Wk=Wk, Wv=Wv, W_out=W_out, scale=scale,
                sv=sv, off=off, se=se)


def make_in_maps(x, W_qkv, W_out, n_tok: int = N_TOK, prep=None):
    p = prep if prep is not None else _prep(x, W_qkv, W_out, n_tok)
    bf = ml_dtypes.bfloat16
    in_maps = []
    for c in range(8):
        b, hp = c // 4, c % 4
        heads = (2 * hp, 2 * hp + 1)
        def hcols(W, h):
            return W[:, h * HD : (h + 1) * HD]
        wq4 = np.concatenate(
            [hcols(p["Wq"], heads[0])] * 2 + [hcols(p["Wq"], heads[1])] * 2,
            axis=1) * p["scale"]
        wk4 = np.concatenate(
            [hcols(p["Wk"], heads[0])] * 2 + [hcols(p["Wk"], heads[1])] * 2,
            axis=1)
        wv = np.zeros((2, 128, 64), dtype=np.float32)
        for ch in range(2):
            rows = slice(ch * 128, (ch + 1) * 128)
            for hi, h in enumerate(heads):
                wv[ch, :, 32 * hi : 32 * hi + 32] = (
                    p["Wv"][rows, h * HD : (h + 1) * HD] * p["sv"]
                )
        # out-proj: row 0 = denominator passthrough, rows 1:33 = W_out/sv
        wo_list = []
        for h in heads:
            aug = np.zeros((33, 258), dtype=np.float32)
            aug[1:33, :256] = p["W_out"][h * HD : (h + 1) * HD, :] / p["sv"]
            aug[0, 256] = 1.0
            wo_list.append(aug)
        in_maps.append(
            {
                "xT": np.ascontiguousarray(p["xf"][b]).astype(bf),
                "wq4": np.ascontiguousarray(wq4).astype(bf),
                "wk4": np.ascontiguousarray(wk4).astype(bf),
                "wv2": wv.astype(bf),
                "wo0": wo_list[0].astype(bf),
                "wo1": wo_list[1].astype(bf),
            }
        )
    return in_maps


def gather(results, x, b_out):
    x = np.asarray(x, dtype=np.float32)
    b_out = np.asarray(b_out, dtype=np.float32)
    Bb, C, D, H, W = x.shape
    out = np.empty_like(x)
    for b in range(Bb):
        acc = results[4 * b]["y"].astype(np.float32)
        for hp in range(1, 4):
            acc = acc + results[4 * b + hp]["y"].astype(np.float32)
        acc += b_out[None, :]
        out[b] = x[b] + acc.T.reshape(C, D, H, W)
    return out


def kernel(x, W_qkv, W_out, b_out):
    x = np.asarray(x, dtype=np.float32)
    b_out = np.asarray(b_out, dtype=np.float32)
    p = _prep(x, W_qkv, W_out, N_TOK)
    nc = build_nc(N_TOK, se=p["se"], off=p["off"])
    in_maps = make_in_maps(x, W_qkv, W_out, N_TOK, prep=p)
    res = run_bass_kernel_spmd(nc, in_maps, list(range(8)))
    return gather(res.results, x, b_out)

